# revision 15
# baseline (speedup 1.0000x reference)
"""Trainium2 Bass kernel for a dense transformer block (DyT-norm causal attention + GELU MLP).

Sharding: 8 cores, SPMD single NEFF. Core c handles batch b=c//4 and query tokens
[qs*512:(qs+1)*512] with qs=c%4. Each core computes K/V projections for the full
sequence of its batch (replicated across the 4 cores of a batch), attention for
its query slice over all 16 heads, then projection + MLP on its token slice.
No collectives: outputs are disjoint token slices, gathered on the host.

Causal masking with a uniform NEFF: the host permutes each core's key/value token
order to [query-window | earlier | later]. KV blocks 0-3 are then always the
diagonal (static triangular mask constants), and the remaining blocks are handled
by a per-core additive bias column (0 = keep, -30000 = drop) applied inside the
softmax exp. Softmax is computed un-shifted (logits are small at init scale), and
the denominator is fused into the attention@V matmul via a ones-column on V.

Matmuls run in float32r (full PE rate at free dim 512) except attention
score/AV matmuls which use bf16 operands with fp32 PSUM accumulation.
"""

import sys
from contextlib import ExitStack

for _p in ('/opt/trn_rl_repo',):
    if _p not in sys.path:
        sys.path.insert(0, _p)

import numpy as np
import ml_dtypes

import concourse.bass as bass
import concourse.mybir as mybir
from concourse.bacc import Bacc
from concourse.bass_utils import run_bass_kernel_spmd
from concourse.tile import TileContext

C = 1024
H = 16
D = 64
FF = 4096
T = 2048
TQ = 512          # query tokens per core
NEG = -30000.0
F32 = mybir.dt.float32
F32R = mybir.dt.float32r
BF16 = mybir.dt.bfloat16
AF = mybir.ActivationFunctionType
ALU = mybir.AluOpType

_CACHE = {}


def _r128(dram_ap):
    """[(m*128), f] DRAM view -> [128, m, f]"""
    return dram_ap.rearrange("(m p) f -> p m f", p=128)


def _build():
    nc = Bacc(trn_type='TRN2')

    # ---- DRAM I/O ----
    xT_d = nc.dram_tensor('xT', [C, T], F32, kind='ExternalInput')
    xqb_d = nc.dram_tensor('xqb', [C, TQ], F32, kind='ExternalInput')
    wq_d = nc.dram_tensor('wq', [C, C], F32R, kind='ExternalInput')
    wk_d = nc.dram_tensor('wk', [C, C], F32R, kind='ExternalInput')
    wv_d = nc.dram_tensor('wv', [C, C], F32R, kind='ExternalInput')
    wproj_d = nc.dram_tensor('wproj', [C, C], F32R, kind='ExternalInput')
    wfc_d = nc.dram_tensor('wfc', [C, FF], F32R, kind='ExternalInput')
    wfc2_d = nc.dram_tensor('wfc2', [FF, C], F32R, kind='ExternalInput')
    bq_d = nc.dram_tensor('bq', [128, 8], F32, kind='ExternalInput')
    bk_d = nc.dram_tensor('bk', [128, 8], F32, kind='ExternalInput')
    bv_d = nc.dram_tensor('bv', [128, C], F32, kind='ExternalInput')
    bfc_d = nc.dram_tensor('bfc', [128, 32], F32, kind='ExternalInput')
    bfc2_d = nc.dram_tensor('bfc2', [128, 8], F32, kind='ExternalInput')
    alpha_d = nc.dram_tensor('alpha_b', [128, 1], F32, kind='ExternalInput')
    gamma_d = nc.dram_tensor('gamma_c', [128, 8], F32, kind='ExternalInput')
    beta_d = nc.dram_tensor('beta_c', [128, 8], F32, kind='ExternalInput')
    mtri_d = nc.dram_tensor('mask_tri', [128, 4, TQ], F32, kind='ExternalInput')
    bcol_d = nc.dram_tensor('bias_cols', [128, 16], F32, kind='ExternalInput')
    ones_d = nc.dram_tensor('ones_bf', [128, 16], BF16, kind='ExternalInput')
    yT_d = nc.dram_tensor('yT', [C, TQ], F32, kind='ExternalOutput')

    with TileContext(nc) as tc, ExitStack() as top:
        cpool = top.enter_context(tc.tile_pool(name='const', bufs=1))

        def cload(shape, dt, dram, tag):
            t = cpool.tile(shape, dt, tag=tag)
            nc.sync.dma_start(t[:], dram[:])
            return t

        alpha_t = cload([128, 1], F32, alpha_d, 'c_alpha')
        gamma_t = cload([128, 8], F32, gamma_d, 'c_gamma')
        beta_t = cload([128, 8], F32, beta_d, 'c_beta')
        bq_t = cload([128, 8], F32, bq_d, 'c_bq')
        bk_t = cload([128, 8], F32, bk_d, 'c_bk')
        bv_t = cload([128, C], F32, bv_d, 'c_bv')
        bfc_t = cload([128, 32], F32, bfc_d, 'c_bfc')
        bfc2_t = cload([128, 8], F32, bfc2_d, 'c_bfc2')
        bcol_t = cload([128, 16], F32, bcol_d, 'c_bcol')
        ones_t = cload([128, 16], BF16, ones_d, 'c_ones')

        xT_r = _r128(xT_d[:])      # [128, 8, 2048]
        xqb_r = _r128(xqb_d[:])    # [128, 8, 512]
        yT_r = _r128(yT_d[:])      # [128, 8, 512]

        # attnT outlives kqv (written in B, read in C); pools pop LIFO so it
        # opens first and closes at TileContext exit.
        attnT = top.enter_context(tc.tile_pool(name='attnT', bufs=1)).tile(
            [128, 8, TQ], F32R)

        # K/Q/V buffers live through phases A+B
        es_kqv = ExitStack()
        kqv = es_kqv.enter_context(tc.tile_pool(name='kqv', bufs=1))
        K_bf = kqv.tile([128, 8, T], BF16)            # K^T
        Q_bf = kqv.tile([128, 8, TQ], BF16)           # Q^T
        V_bf = kqv.tile([128, 16, H, D + 1], BF16)    # token-major V + ones col

        # ================= Phase A: DyT + QKV projections =================
        with (
            tc.tile_pool(name='hT_pool', bufs=1) as hpool,
            tc.tile_pool(name='stageA', bufs=2) as spool,
            tc.tile_pool(name='wA', bufs=8) as wpool,
            tc.tile_pool(name='wvA', bufs=1) as wvpool,
            tc.tile_pool(name='psA', bufs=4, space='PSUM') as psA,
        ):
            hT = hpool.tile([128, 8, T], F32R)
            # DyT: hT = gamma * tanh(alpha * x) + beta
            for kt in range(8):
                for nt in range(4):
                    xt = spool.tile([128, TQ], F32, tag='xstage')
                    nc.sync.dma_start(xt[:], xT_r[:, kt, nt * TQ:(nt + 1) * TQ])
                    tmp = spool.tile([128, TQ], F32, tag='tanh')
                    nc.scalar.activation(tmp[:], xt[:], AF.Tanh, scale=alpha_t[:, 0:1])
                    nc.vector.tensor_scalar(
                        hT[:, kt, nt * TQ:(nt + 1) * TQ], tmp[:],
                        gamma_t[:, kt:kt + 1], beta_t[:, kt:kt + 1],
                        ALU.mult, ALU.add)

            wk_r = _r128(wk_d[:])    # [128, 8, 1024]
            wq_r = _r128(wq_d[:])
            wv_r = _r128(wv_d[:])

            # K^T = wk^T @ hT  (+bk)
            for mt in range(8):
                wts = []
                for kt in range(8):
                    wt = wpool.tile([128, 128], F32R, tag='wkq')
                    nc.sync.dma_start(wt[:], wk_r[:, kt, mt * 128:(mt + 1) * 128])
                    wts.append(wt)
                for nt in range(4):
                    ps = psA.tile([128, TQ], F32)
                    for kt in range(8):
                        nc.tensor.matmul(ps[:], wts[kt][:], hT[:, kt, nt * TQ:(nt + 1) * TQ],
                                         start=(kt == 0), stop=(kt == 7))
                    nc.scalar.activation(K_bf[:, mt, nt * TQ:(nt + 1) * TQ], ps[:],
                                         AF.Identity, bias=bk_t[:, mt:mt + 1])

            # Q^T = wq^T @ hT[:, :512]  (+bq)
            for mt in range(8):
                wts = []
                for kt in range(8):
                    wt = wpool.tile([128, 128], F32R, tag='wkq')
                    nc.sync.dma_start(wt[:], wq_r[:, kt, mt * 128:(mt + 1) * 128])
                    wts.append(wt)
                ps = psA.tile([128, TQ], F32)
                for kt in range(8):
                    nc.tensor.matmul(ps[:], wts[kt][:], hT[:, kt, 0:TQ],
                                     start=(kt == 0), stop=(kt == 7))
                nc.scalar.activation(Q_bf[:, mt, :], ps[:], AF.Identity,
                                     bias=bq_t[:, mt:mt + 1])

            # V = hT^T @ wv (token-major) (+bv), into [128, kvb, head, 65] with ones col
            for n2 in range(2):
                wvt = wvpool.tile([128, 8, TQ], F32R, tag='wv')
                nc.sync.dma_start(wvt[:], wv_r[:, :, n2 * TQ:(n2 + 1) * TQ])
                for kvb in range(16):
                    ps = psA.tile([128, TQ], F32)
                    for kt in range(8):
                        nc.tensor.matmul(ps[:], hT[:, kt, kvb * 128:(kvb + 1) * 128],
                                         wvt[:, kt, :],
                                         start=(kt == 0), stop=(kt == 7))
                    bvb = bv_t[:, n2 * TQ:(n2 + 1) * TQ].rearrange(
                        "p (h d) -> p h d", d=D)
                    nc.vector.tensor_tensor(
                        V_bf[:, kvb, n2 * 8:(n2 + 1) * 8, 0:D],
                        ps[:].rearrange("p (h d) -> p h d", d=D),
                        bvb, ALU.add)
            for kvb in range(16):
                nc.vector.tensor_copy(V_bf[:, kvb, :, D], ones_t[:, :])

        # ================= Phase B: attention =================
        with (
            tc.tile_pool(name='pB', bufs=4) as pbpool,
            tc.tile_pool(name='mtriB', bufs=1) as mtpool,
            tc.tile_pool(name='psS', bufs=4, space='PSUM') as psS,
            tc.tile_pool(name='psO', bufs=2, space='PSUM') as psO,
        ):
            mtri_t = mtpool.tile([128, 4, TQ], F32)
            nc.sync.dma_start(mtri_t[:], mtri_d[:])
            for h in range(H):
                hb = (h % 2) * 64
                hc = h // 2
                po = psO.tile([65, TQ], F32, tag='po')
                for kvb in range(16):
                    ps = psS.tile([128, TQ], F32, tag='score')
                    nc.tensor.matmul(ps[:],
                                     K_bf[hb:hb + 64, hc, kvb * 128:(kvb + 1) * 128],
                                     Q_bf[hb:hb + 64, hc, :],
                                     start=True, stop=True)
                    if kvb < 4:
                        nc.vector.tensor_tensor(ps[:], ps[:], mtri_t[:, kvb, :], ALU.add)
                    pt = pbpool.tile([128, TQ], BF16, tag='probs')
                    nc.scalar.activation(pt[:], ps[:], AF.Exp,
                                         bias=bcol_t[:, kvb:kvb + 1], scale=0.125)
                    nc.tensor.matmul(po[:], V_bf[:, kvb, h, :], pt[:],
                                     start=(kvb == 0), stop=(kvb == 15))
                rec = pbpool.tile([1, TQ], F32, tag='recip')
                nc.vector.reciprocal(rec[:], po[64:65, :])
                rec64 = pbpool.tile([64, TQ], F32, tag='recip64')
                nc.gpsimd.partition_broadcast(rec64[:], rec[0:1, :])
                nc.vector.tensor_tensor(attnT[hb:hb + 64, hc, :], po[0:64, :],
                                        rec64[:], ALU.mult)
        es_kqv.close()

        # x2T/h2T live through phases C+D
        es_mlp = ExitStack()
        mpool = es_mlp.enter_context(tc.tile_pool(name='mlp', bufs=1))
        x2T = mpool.tile([128, 8, TQ], F32)
        h2T = mpool.tile([128, 8, TQ], F32R)

        # ================= Phase C: proj + residual + DyT2 =================
        with (
            tc.tile_pool(name='stageC', bufs=3) as scpool,
            tc.tile_pool(name='xqbC', bufs=1) as xqpool,
            tc.tile_pool(name='wC', bufs=16) as wcpool,
            tc.tile_pool(name='psC', bufs=4, space='PSUM') as psC,
        ):
            xqb_t = xqpool.tile([128, 8, TQ], F32)
            nc.sync.dma_start(xqb_t[:], xqb_r[:])
            wproj_r = _r128(wproj_d[:])
            for mt in range(8):
                wts = []
                for kt in range(8):
                    wt = wcpool.tile([128, 128], F32R, tag='wproj')
                    nc.sync.dma_start(wt[:], wproj_r[:, kt, mt * 128:(mt + 1) * 128])
                    wts.append(wt)
                ps = psC.tile([128, TQ], F32)
                for kt in range(8):
                    nc.tensor.matmul(ps[:], wts[kt][:], attnT[:, kt, :],
                                     start=(kt == 0), stop=(kt == 7))
                nc.vector.tensor_tensor(x2T[:, mt, :], ps[:], xqb_t[:, mt, :], ALU.add)
                tmp = scpool.tile([128, TQ], F32, tag='tanh2')
                nc.scalar.activation(tmp[:], x2T[:, mt, :], AF.Tanh, scale=alpha_t[:, 0:1])
                nc.vector.tensor_scalar(h2T[:, mt, :], tmp[:],
                                        gamma_t[:, mt:mt + 1], beta_t[:, mt:mt + 1],
                                        ALU.mult, ALU.add)

        # ================= Phase D: MLP =================
        with (
            tc.tile_pool(name='gT_pool', bufs=1) as gpool,
            tc.tile_pool(name='stageD', bufs=3) as sdpool,
            tc.tile_pool(name='wD', bufs=16) as wdpool,
            tc.tile_pool(name='psD', bufs=4, space='PSUM') as psD,
        ):
            gT = gpool.tile([128, 32, TQ], F32R)
            wfc_r = _r128(wfc_d[:])     # [128, 8, 4096]
            for mt in range(32):
                wts = []
                for kt in range(8):
                    wt = wdpool.tile([128, 128], F32R, tag='wfc')
                    nc.sync.dma_start(wt[:], wfc_r[:, kt, mt * 128:(mt + 1) * 128])
                    wts.append(wt)
                ps = psD.tile([128, TQ], F32)
                for kt in range(8):
                    nc.tensor.matmul(ps[:], wts[kt][:], h2T[:, kt, :],
                                     start=(kt == 0), stop=(kt == 7))
                nc.scalar.activation(gT[:, mt, :], ps[:], AF.Gelu,
                                     bias=bfc_t[:, mt:mt + 1])

            wfc2_r = _r128(wfc2_d[:])   # [128, 32, 1024]
            for mt in range(8):
                wts = []
                for kt in range(32):
                    wt = wdpool.tile([128, 128], F32R, tag='wfc2')
                    nc.sync.dma_start(wt[:], wfc2_r[:, kt, mt * 128:(mt + 1) * 128])
                    wts.append(wt)
                ps = psD.tile([128, TQ], F32)
                for kt in range(32):
                    nc.tensor.matmul(ps[:], wts[kt][:], gT[:, kt, :],
                                     start=(kt == 0), stop=(kt == 31))
                tmp = sdpool.tile([128, TQ], F32, tag='bias2')
                nc.vector.tensor_scalar(tmp[:], ps[:], bfc2_t[:, mt:mt + 1], None, ALU.add)
                yt = sdpool.tile([128, TQ], F32, tag='yout')
                nc.vector.tensor_tensor(yt[:], tmp[:], x2T[:, mt, :], ALU.add)
                nc.sync.dma_start(yT_r[:, mt, :], yt[:])
        es_mlp.close()

    nc.finalize()
    return nc


def _prep_inputs(x, alpha, gamma, beta, w_attn, b_attn, w_proj, b_proj,
                 w_fc, b_fc, w_fc2, b_fc2):
    f = np.float32
    wq = np.ascontiguousarray(w_attn[:, :C], f)
    wk = np.ascontiguousarray(w_attn[:, C:2 * C], f)
    wv = np.ascontiguousarray(w_attn[:, 2 * C:], f)
    bq = np.ascontiguousarray(b_attn[:C].reshape(8, 128).T, f)
    bk = np.ascontiguousarray(b_attn[C:2 * C].reshape(8, 128).T, f)
    bv = np.ascontiguousarray(np.tile(b_attn[2 * C:].reshape(1, C), (128, 1)), f)
    bfc = np.ascontiguousarray(b_fc.reshape(32, 128).T, f)
    bfc2 = np.ascontiguousarray(b_fc2.reshape(8, 128).T, f)
    alpha_b = np.full((128, 1), float(np.asarray(alpha).reshape(-1)[0]), f)
    gamma_c = np.ascontiguousarray(np.asarray(gamma, f).reshape(8, 128).T, f)
    beta_c = np.ascontiguousarray(np.asarray(beta, f).reshape(8, 128).T, f)
    r = np.arange(128)[:, None, None]
    tt = np.arange(4)[None, :, None]
    p = np.arange(TQ)[None, None, :]
    mask_tri = np.where(tt * 128 + r <= p, 0.0, NEG).astype(f)
    ones_bf = np.ones((128, 16), ml_dtypes.bfloat16)

    shared = dict(wq=wq, wk=wk, wv=wv, wproj=np.ascontiguousarray(w_proj, f),
                  wfc=np.ascontiguousarray(w_fc, f),
                  wfc2=np.ascontiguousarray(w_fc2, f),
                  bq=bq, bk=bk, bv=bv, bfc=bfc, bfc2=bfc2,
                  alpha_b=alpha_b, gamma_c=gamma_c, beta_c=beta_c,
                  mask_tri=mask_tri, ones_bf=ones_bf)

    in_maps = []
    for c in range(8):
        b, qs = c // 4, c % 4
        perm = np.concatenate([np.arange(qs * TQ, (qs + 1) * TQ),
                               np.arange(0, qs * TQ),
                               np.arange((qs + 1) * TQ, T)])
        xT = np.ascontiguousarray(np.asarray(x[b], f).T[:, perm])
        xqb = np.ascontiguousarray(np.asarray(x[b, qs * TQ:(qs + 1) * TQ], f).T
                                   + np.asarray(b_proj, f)[:, None])
        bias_cols = np.zeros((128, 16), f)
        bias_cols[:, 4 + 4 * qs:] = NEG
        in_maps.append(dict(shared, xT=xT, xqb=xqb, bias_cols=bias_cols))
    return in_maps


def kernel(**inputs):
    if 'nc' not in _CACHE:
        _CACHE['nc'] = _build()
    nc = _CACHE['nc']
    in_maps = _prep_inputs(**inputs)
    res = run_bass_kernel_spmd(nc, in_maps, core_ids=list(range(8)))
    out = np.zeros((2, T, C), np.float32)
    for c in range(8):
        b, qs = c // 4, c % 4
        out[b, qs * TQ:(qs + 1) * TQ, :] = res.results[c]['yT'].T
    return out


# revision 43
# speedup vs baseline: 1.0104x; 1.0104x over previous
"""Trainium2 Bass kernel for a dense transformer block (DyT-norm causal attention + GELU MLP).

Sharding: 8 cores, SPMD single NEFF. Core c handles batch b=c//4 and query tokens
[qs*512:(qs+1)*512] with qs=c%4. Each core computes K/V projections for the full
sequence of its batch (replicated across the 4 cores of a batch), attention for
its query slice over all 16 heads, then projection + MLP on its token slice.
No collectives: outputs are disjoint token slices, gathered on the host.

Causal masking with a uniform NEFF: the host permutes each core's key/value token
order to [query-window | earlier | later]. KV blocks 0-3 are then always the
diagonal (static triangular mask constants), and the remaining blocks are handled
by a per-core additive bias column (0 = keep, -30000 = drop) applied inside the
softmax exp. Softmax is computed un-shifted (logits are small at init scale), and
the denominator is fused into the attention@V matmul via a ones-column on V.

Matmuls run in float32r (full PE rate at free dim 512) except attention
score/AV matmuls which use bf16 operands with fp32 PSUM accumulation.
"""

import sys
from contextlib import ExitStack

for _p in ('/opt/trn_rl_repo',):
    if _p not in sys.path:
        sys.path.insert(0, _p)

import numpy as np
import ml_dtypes

import concourse.bass as bass
import concourse.mybir as mybir
from concourse.bacc import Bacc
from concourse.bass_utils import run_bass_kernel_spmd
from concourse.tile import TileContext

C = 1024
H = 16
D = 64
FF = 4096
T = 2048
TQ = 512          # query tokens per core
NEG = -30000.0
F32 = mybir.dt.float32
F32R = mybir.dt.float32r
BF16 = mybir.dt.bfloat16
AF = mybir.ActivationFunctionType
ALU = mybir.AluOpType

_CACHE = {}


def _r128(dram_ap):
    """[(m*128), f] DRAM view -> [128, m, f]"""
    return dram_ap.rearrange("(m p) f -> p m f", p=128)


def _build(phases='ABCD'):
    nc = Bacc(trn_type='TRN2')

    # ---- DRAM I/O ----
    xT_d = nc.dram_tensor('xT', [C, T], F32, kind='ExternalInput')
    xqb_d = nc.dram_tensor('xqb', [C, TQ], F32, kind='ExternalInput')
    # Weights are host-pretiled to [128, mt, kt, 128] so each matmul group's
    # lhsT tiles arrive in ONE contiguous-per-partition DMA.
    wq_d = nc.dram_tensor('wq', [128, 8, 8, 128], F32R, kind='ExternalInput')
    wk_d = nc.dram_tensor('wk', [128, 8, 8, 128], F32R, kind='ExternalInput')
    wv_d = nc.dram_tensor('wv', [C, C], F32R, kind='ExternalInput')
    wproj_d = nc.dram_tensor('wproj', [128, 8, 8, 128], F32R, kind='ExternalInput')
    wfc_d = nc.dram_tensor('wfc', [128, 32, 8, 128], F32R, kind='ExternalInput')
    wfc2_d = nc.dram_tensor('wfc2', [128, 8, 32, 128], F32R, kind='ExternalInput')
    bq_d = nc.dram_tensor('bq', [128, 8], F32, kind='ExternalInput')
    bk_d = nc.dram_tensor('bk', [128, 8], F32, kind='ExternalInput')
    bv_d = nc.dram_tensor('bv', [128, C], F32, kind='ExternalInput')
    bfc_d = nc.dram_tensor('bfc', [128, 32], F32, kind='ExternalInput')
    bfc2_d = nc.dram_tensor('bfc2', [128, 8], F32, kind='ExternalInput')
    alpha_d = nc.dram_tensor('alpha_b', [128, 1], F32, kind='ExternalInput')
    gamma_d = nc.dram_tensor('gamma_c', [128, 8], F32, kind='ExternalInput')
    beta_d = nc.dram_tensor('beta_c', [128, 8], F32, kind='ExternalInput')
    mtri_d = nc.dram_tensor('mask_tri', [128, 4, TQ], F32, kind='ExternalInput')
    bcol_d = nc.dram_tensor('bias_cols', [128, 8], F32, kind='ExternalInput')
    ones_d = nc.dram_tensor('ones_bf', [128, 16], BF16, kind='ExternalInput')
    yT_d = nc.dram_tensor('yT', [C, TQ], F32, kind='ExternalOutput')

    with TileContext(nc) as tc, ExitStack() as top:
        cpool = top.enter_context(tc.tile_pool(name='const', bufs=1))

        def cload(shape, dt, dram, tag):
            t = cpool.tile(shape, dt, tag=tag)
            nc.sync.dma_start(t[:], dram[:])
            return t

        alpha_t = cload([128, 1], F32, alpha_d, 'c_alpha')
        gamma_t = cload([128, 8], F32, gamma_d, 'c_gamma')
        beta_t = cload([128, 8], F32, beta_d, 'c_beta')
        bq_t = cload([128, 8], F32, bq_d, 'c_bq')
        bk_t = cload([128, 8], F32, bk_d, 'c_bk')
        bv_t = cload([128, C], F32, bv_d, 'c_bv')
        bfc_t = cload([128, 32], F32, bfc_d, 'c_bfc')
        bfc2_t = cload([128, 8], F32, bfc2_d, 'c_bfc2')
        bcol2_t = cload([128, 8], F32, bcol_d, 'c_bcol')
        ones_t = cload([128, 16], BF16, ones_d, 'c_ones')

        xT_r = _r128(xT_d[:])      # [128, 8, 2048]
        xqb_r = _r128(xqb_d[:])    # [128, 8, 512]
        yT_r = _r128(yT_d[:])      # [128, 8, 512]

        # attnT outlives kqv (written in B, read in C); pools pop LIFO so it
        # opens first and closes at TileContext exit. Tile created lazily at
        # first use (phase B) so it doesn't occupy SBUF during phase A.
        attnT_pool = top.enter_context(tc.tile_pool(name='attnT', bufs=1))

        # K/Q/V buffers live through phases A+B
        es_kqv = ExitStack()
        kqv = es_kqv.enter_context(tc.tile_pool(name='kqv', bufs=1))
        K_bf = kqv.tile([128, 8, T], BF16)            # K^T
        Q_bf = kqv.tile([128, 8, TQ], BF16)           # Q^T
        V_bf = kqv.tile([128, 16, H, D + 1], BF16)    # token-major V + ones col

        # ================= Phase A: DyT + QKV projections =================
        with (
            tc.tile_pool(name='hT_pool', bufs=1) as hpool,
            tc.tile_pool(name='stageA', bufs=2) as spool,
            tc.tile_pool(name='wA', bufs=8) as wpool,
            tc.tile_pool(name='wvA', bufs=1) as wvpool,
            tc.tile_pool(name='psA', bufs=4, space='PSUM') as psA,
        ):
            hT = hpool.tile([128, 8, T], F32R)
            # DyT with gamma/beta folded into the weights host-side:
            # hT = tanh(alpha * x), batched 4 kt-chunks per op.
            # nt-outer so K-proj's first (mt, nt=0) group unblocks early.
            for nt in range(4):
                for k4 in range(2):
                    xt = spool.tile([128, 4, TQ], F32, tag='xstage')
                    nc.sync.dma_start(
                        xt[:], xT_r[:, k4 * 4:(k4 + 1) * 4, nt * TQ:(nt + 1) * TQ])
                    nc.scalar.activation(
                        hT[:, k4 * 4:(k4 + 1) * 4, nt * TQ:(nt + 1) * TQ],
                        xt[:], AF.Tanh, scale=alpha_t[:, 0:1])

            wv_r = _r128(wv_d[:])

            # K^T = wk^T @ hT  (+bk)
            for mt in range(8):
                wt = wpool.tile([128, 8, 128], F32R, tag='wkq')
                nc.sync.dma_start(wt[:], wk_d[:, mt])
                for nt in range(4):
                    ps = psA.tile([128, TQ], F32)
                    for kt in range(8):
                        nc.tensor.matmul(ps[:], wt[:, kt, :], hT[:, kt, nt * TQ:(nt + 1) * TQ],
                                         start=(kt == 0), stop=(kt == 7))
                    nc.scalar.activation(K_bf[:, mt, nt * TQ:(nt + 1) * TQ], ps[:],
                                         AF.Identity, bias=bk_t[:, mt:mt + 1])

            # Q^T = wq^T @ hT[:, :512]  (+bq)
            for mt in range(8):
                wt = wpool.tile([128, 8, 128], F32R, tag='wkq')
                nc.sync.dma_start(wt[:], wq_d[:, mt])
                ps = psA.tile([128, TQ], F32)
                for kt in range(8):
                    nc.tensor.matmul(ps[:], wt[:, kt, :], hT[:, kt, 0:TQ],
                                     start=(kt == 0), stop=(kt == 7))
                nc.scalar.activation(Q_bf[:, mt, :], ps[:], AF.Identity,
                                     bias=bq_t[:, mt:mt + 1])

            # V = hT^T @ wv (token-major) (+bv), into [128, kvb, head, 65] with ones col
            for n2 in range(2):
                wvt = wvpool.tile([128, 8, TQ], F32R, tag='wv')
                nc.sync.dma_start(wvt[:], wv_r[:, :, n2 * TQ:(n2 + 1) * TQ])
                for kvb in range(16):
                    ps = psA.tile([128, TQ], F32)
                    for kt in range(8):
                        nc.tensor.matmul(ps[:], hT[:, kt, kvb * 128:(kvb + 1) * 128],
                                         wvt[:, kt, :],
                                         start=(kt == 0), stop=(kt == 7))
                    bvb = bv_t[:, n2 * TQ:(n2 + 1) * TQ].rearrange(
                        "p (h d) -> p h d", d=D)
                    nc.vector.tensor_tensor(
                        V_bf[:, kvb, n2 * 8:(n2 + 1) * 8, 0:D],
                        ps[:].rearrange("p (h d) -> p h d", d=D),
                        bvb, ALU.add)
            for kvb in range(16):
                nc.vector.tensor_copy(V_bf[:, kvb, :, D], ones_t[:, :])

        # ================= Phase B: attention =================
        with (
            tc.tile_pool(name='pB', bufs=4) as pbpool,
            tc.tile_pool(name='mtriB', bufs=1) as mtpool,
            tc.tile_pool(name='psS', bufs=3, space='PSUM') as psS,
            tc.tile_pool(name='psO', bufs=2, space='PSUM') as psO,
        ):
            mtri_t = mtpool.tile([128, 4, TQ], F32)
            nc.sync.dma_start(mtri_t[:], mtri_d[:])
            attnT = attnT_pool.tile([128, 8, TQ], F32R)
            for h in range(H if 'B' in phases else 0):
                hb = (h % 2) * 64
                hc = h // 2
                po = psO.tile([65, TQ], F32, tag='po')
                for kv2 in range(8):
                    # two kv blocks share one PSUM tile so exp runs [128, 1024]
                    ps = psS.tile([128, 2, TQ], F32, tag='score')
                    pt = pbpool.tile([128, 2, TQ], BF16, tag='probs')
                    for j in range(2):
                        kvb = kv2 * 2 + j
                        nc.tensor.matmul(ps[:, j, :],
                                         K_bf[hb:hb + 64, hc, kvb * 128:(kvb + 1) * 128],
                                         Q_bf[hb:hb + 64, hc, :],
                                         start=True, stop=True)
                        if kvb < 4:
                            nc.vector.tensor_tensor(ps[:, j, :], ps[:, j, :],
                                                    mtri_t[:, kvb, :], ALU.add)
                    nc.scalar.activation(
                        pt[:], ps[:], AF.Exp,
                        bias=bcol2_t[:, kv2:kv2 + 1], scale=0.125)
                    for j in range(2):
                        kvb = kv2 * 2 + j
                        nc.tensor.matmul(po[:], V_bf[:, kvb, h, :], pt[:, j, :],
                                         start=(kvb == 0), stop=(kvb == 15))
                rec = pbpool.tile([1, TQ], F32, tag='recip')
                nc.vector.reciprocal(rec[:], po[64:65, :])
                rec64 = pbpool.tile([64, TQ], F32, tag='recip64')
                nc.gpsimd.partition_broadcast(rec64[:], rec[0:1, :])
                nc.vector.tensor_tensor(attnT[hb:hb + 64, hc, :], po[0:64, :],
                                        rec64[:], ALU.mult)
        es_kqv.close()

        # x2T/h2T live through phases C+D
        es_mlp = ExitStack()
        mpool = es_mlp.enter_context(tc.tile_pool(name='mlp', bufs=1))
        x2T = mpool.tile([128, 8, TQ], F32)
        h2T = mpool.tile([128, 8, TQ], F32R)

        # ================= Phase C: proj + residual + DyT2 =================
        with (
            tc.tile_pool(name='stageC', bufs=3) as scpool,
            tc.tile_pool(name='xqbC', bufs=1) as xqpool,
            tc.tile_pool(name='wC', bufs=16) as wcpool,
            tc.tile_pool(name='psC', bufs=4, space='PSUM') as psC,
        ):
            xqb_t = xqpool.tile([128, 8, TQ], F32)
            nc.sync.dma_start(xqb_t[:], xqb_r[:])
            for mt in range(8 if 'C' in phases else 0):
                wt = wcpool.tile([128, 8, 128], F32R, tag='wproj')
                nc.sync.dma_start(wt[:], wproj_d[:, mt])
                ps = psC.tile([128, TQ], F32)
                for kt in range(8):
                    nc.tensor.matmul(ps[:], wt[:, kt, :], attnT[:, kt, :],
                                     start=(kt == 0), stop=(kt == 7))
                nc.vector.tensor_tensor(x2T[:, mt, :], ps[:], xqb_t[:, mt, :], ALU.add)
                nc.scalar.activation(h2T[:, mt, :], x2T[:, mt, :], AF.Tanh,
                                     scale=alpha_t[:, 0:1])

        # ================= Phase D: MLP =================
        with (
            tc.tile_pool(name='gT_pool', bufs=1) as gpool,
            tc.tile_pool(name='stageD', bufs=3) as sdpool,
            tc.tile_pool(name='wD', bufs=16) as wdpool,
            tc.tile_pool(name='psD', bufs=4, space='PSUM') as psD,
        ):
            gT = gpool.tile([128, 32, TQ], F32R)
            for mt in range(32 if 'D' in phases else 0):
                wt = wdpool.tile([128, 8, 128], F32R, tag='wfc')
                nc.sync.dma_start(wt[:], wfc_d[:, mt])
                ps = psD.tile([128, TQ], F32)
                for kt in range(8):
                    nc.tensor.matmul(ps[:], wt[:, kt, :], h2T[:, kt, :],
                                     start=(kt == 0), stop=(kt == 7))
                nc.scalar.activation(gT[:, mt, :], ps[:], AF.Gelu,
                                     bias=bfc_t[:, mt:mt + 1])

            for mt in range(8 if 'D' in phases else 0):
                wt = wdpool.tile([128, 32, 128], F32R, tag='wfc2')
                nc.sync.dma_start(wt[:], wfc2_d[:, mt])
                ps = psD.tile([128, TQ], F32)
                for kt in range(32):
                    nc.tensor.matmul(ps[:], wt[:, kt, :], gT[:, kt, :],
                                     start=(kt == 0), stop=(kt == 31))
                tmp = sdpool.tile([128, TQ], F32, tag='bias2')
                nc.vector.tensor_scalar(tmp[:], ps[:], bfc2_t[:, mt:mt + 1], None, ALU.add)
                yt = sdpool.tile([128, TQ], F32, tag='yout')
                nc.vector.tensor_tensor(yt[:], tmp[:], x2T[:, mt, :], ALU.add)
                nc.sync.dma_start(yT_r[:, mt, :], yt[:])
        es_mlp.close()

    nc.finalize()
    return nc


def _prep_inputs(x, alpha, gamma, beta, w_attn, b_attn, w_proj, b_proj,
                 w_fc, b_fc, w_fc2, b_fc2):
    f = np.float32

    def tile_w(w, n_mt):
        # [K, M] -> [128, mt, kt, 128]: element [p, mt, kt, c] = w[kt*128+p, mt*128+c]
        kk, mm = w.shape
        return np.ascontiguousarray(
            np.asarray(w, f).reshape(kk // 128, 128, n_mt, 128).transpose(1, 2, 0, 3))

    # Fold DyT's gamma/beta into the consuming weights:
    #   w.T @ (g*t + b) = (g[:,None]*w).T @ t + (w.T @ b)
    g64 = np.asarray(gamma, np.float64)
    b64 = np.asarray(beta, np.float64)
    w64 = np.asarray(w_attn, np.float64)
    wfc64 = np.asarray(w_fc, np.float64)
    wq64, wk64, wv64 = w64[:, :C], w64[:, C:2 * C], w64[:, 2 * C:]
    bq_e = np.asarray(b_attn[:C], np.float64) + wq64.T @ b64
    bk_e = np.asarray(b_attn[C:2 * C], np.float64) + wk64.T @ b64
    bv_e = np.asarray(b_attn[2 * C:], np.float64) + wv64.T @ b64
    bfc_e = np.asarray(b_fc, np.float64) + wfc64.T @ b64

    wq = tile_w(wq64 * g64[:, None], 8)
    wk = tile_w(wk64 * g64[:, None], 8)
    wv = np.ascontiguousarray(wv64 * g64[:, None], f)
    bq = np.ascontiguousarray(bq_e.reshape(8, 128).T, f)
    bk = np.ascontiguousarray(bk_e.reshape(8, 128).T, f)
    bv = np.ascontiguousarray(np.tile(bv_e.reshape(1, C), (128, 1)), f)
    bfc = np.ascontiguousarray(bfc_e.reshape(32, 128).T, f)
    bfc2 = np.ascontiguousarray(b_fc2.reshape(8, 128).T, f)
    alpha_b = np.full((128, 1), float(np.asarray(alpha).reshape(-1)[0]), f)
    gamma_c = np.ascontiguousarray(np.asarray(gamma, f).reshape(8, 128).T, f)
    beta_c = np.ascontiguousarray(np.asarray(beta, f).reshape(8, 128).T, f)
    r = np.arange(128)[:, None, None]
    tt = np.arange(4)[None, :, None]
    p = np.arange(TQ)[None, None, :]
    mask_tri = np.where(tt * 128 + r <= p, 0.0, NEG).astype(f)
    ones_bf = np.ones((128, 16), ml_dtypes.bfloat16)

    shared = dict(wq=wq, wk=wk, wv=wv, wproj=tile_w(w_proj, 8),
                  wfc=tile_w(wfc64 * g64[:, None], 32),
                  wfc2=tile_w(w_fc2, 8),
                  bq=bq, bk=bk, bv=bv, bfc=bfc, bfc2=bfc2,
                  alpha_b=alpha_b, gamma_c=gamma_c, beta_c=beta_c,
                  mask_tri=mask_tri, ones_bf=ones_bf)

    in_maps = []
    for c in range(8):
        b, qs = c // 4, c % 4
        perm = np.concatenate([np.arange(qs * TQ, (qs + 1) * TQ),
                               np.arange(0, qs * TQ),
                               np.arange((qs + 1) * TQ, T)])
        xT = np.ascontiguousarray(np.asarray(x[b], f).T[:, perm])
        xqb = np.ascontiguousarray(np.asarray(x[b, qs * TQ:(qs + 1) * TQ], f).T
                                   + np.asarray(b_proj, f)[:, None])
        bias_cols = np.zeros((128, 8), f)
        bias_cols[:, 2 + 2 * qs:] = NEG
        in_maps.append(dict(shared, xT=xT, xqb=xqb, bias_cols=bias_cols))
    return in_maps


def kernel(**inputs):
    if 'nc' not in _CACHE:
        _CACHE['nc'] = _build()
    nc = _CACHE['nc']
    in_maps = _prep_inputs(**inputs)
    res = run_bass_kernel_spmd(nc, in_maps, core_ids=list(range(8)))
    out = np.zeros((2, T, C), np.float32)
    for c in range(8):
        b, qs = c // 4, c % 4
        out[b, qs * TQ:(qs + 1) * TQ, :] = res.results[c]['yT'].T
    return out


# revision 44
# speedup vs baseline: 1.0308x; 1.0202x over previous
"""Trainium2 Bass kernel for a dense transformer block (DyT-norm causal attention + GELU MLP).

Sharding: 8 cores, SPMD single NEFF. Core c handles batch b=c//4 and query tokens
[qs*512:(qs+1)*512] with qs=c%4. Each core computes K/V projections for the full
sequence of its batch (replicated across the 4 cores of a batch), attention for
its query slice over all 16 heads, then projection + MLP on its token slice.
No collectives: outputs are disjoint token slices, gathered on the host.

Causal masking with a uniform NEFF: the host permutes each core's key/value token
order to [query-window | earlier | later]. KV blocks 0-3 are then always the
diagonal (static triangular mask constants), and the remaining blocks are handled
by a per-core additive bias column (0 = keep, -30000 = drop) applied inside the
softmax exp. Softmax is computed un-shifted (logits are small at init scale), and
the denominator is fused into the attention@V matmul via a ones-column on V.

Matmuls run in float32r (full PE rate at free dim 512) except attention
score/AV matmuls which use bf16 operands with fp32 PSUM accumulation.
"""

import sys
from contextlib import ExitStack

for _p in ('/opt/trn_rl_repo',):
    if _p not in sys.path:
        sys.path.insert(0, _p)

import numpy as np
import ml_dtypes

import concourse.bass as bass
import concourse.mybir as mybir
from concourse.bacc import Bacc
from concourse.bass_utils import run_bass_kernel_spmd
from concourse.tile import TileContext

C = 1024
H = 16
D = 64
FF = 4096
T = 2048
TQ = 512          # query tokens per core
NEG = -30000.0
F32 = mybir.dt.float32
F32R = mybir.dt.float32r
BF16 = mybir.dt.bfloat16
AF = mybir.ActivationFunctionType
ALU = mybir.AluOpType

_CACHE = {}


def _r128(dram_ap):
    """[(m*128), f] DRAM view -> [128, m, f]"""
    return dram_ap.rearrange("(m p) f -> p m f", p=128)


def _build(phases='ABCD'):
    nc = Bacc(trn_type='TRN2')

    # ---- DRAM I/O ----
    xT_d = nc.dram_tensor('xT', [C, T], F32, kind='ExternalInput')
    xqb_d = nc.dram_tensor('xqb', [C, TQ], F32, kind='ExternalInput')
    # Weights are host-pretiled to [128, mt, kt, 128] so each matmul group's
    # lhsT tiles arrive in ONE contiguous-per-partition DMA.
    wq_d = nc.dram_tensor('wq', [128, 8, 8, 128], F32R, kind='ExternalInput')
    wk_d = nc.dram_tensor('wk', [128, 8, 8, 128], F32R, kind='ExternalInput')
    wv_d = nc.dram_tensor('wv', [C, C], F32R, kind='ExternalInput')
    wproj_d = nc.dram_tensor('wproj', [128, 8, 8, 128], F32R, kind='ExternalInput')
    wfc_d = nc.dram_tensor('wfc', [128, 32, 8, 128], F32R, kind='ExternalInput')
    wfc2_d = nc.dram_tensor('wfc2', [128, 8, 32, 128], F32R, kind='ExternalInput')
    bq_d = nc.dram_tensor('bq', [128, 8], F32, kind='ExternalInput')
    bk_d = nc.dram_tensor('bk', [128, 8], F32, kind='ExternalInput')
    bv_d = nc.dram_tensor('bv', [128, C], F32, kind='ExternalInput')
    bfc_d = nc.dram_tensor('bfc', [128, 32], F32, kind='ExternalInput')
    bfc2_d = nc.dram_tensor('bfc2', [128, 8], F32, kind='ExternalInput')
    alpha_d = nc.dram_tensor('alpha_b', [128, 1], F32, kind='ExternalInput')
    gamma_d = nc.dram_tensor('gamma_c', [128, 8], F32, kind='ExternalInput')
    beta_d = nc.dram_tensor('beta_c', [128, 8], F32, kind='ExternalInput')
    mtri_d = nc.dram_tensor('mask_tri', [128, 4, TQ], F32, kind='ExternalInput')
    bcol_d = nc.dram_tensor('bias_cols', [128, 8], F32, kind='ExternalInput')
    ones_d = nc.dram_tensor('ones_bf', [128, 16], BF16, kind='ExternalInput')
    yT_d = nc.dram_tensor('yT', [C, TQ], F32, kind='ExternalOutput')

    with TileContext(nc) as tc, ExitStack() as top:
        cpool = top.enter_context(tc.tile_pool(name='const', bufs=1))

        def cload(shape, dt, dram, tag):
            t = cpool.tile(shape, dt, tag=tag)
            nc.gpsimd.dma_start(t[:], dram[:])
            return t

        alpha_t = cload([128, 1], F32, alpha_d, 'c_alpha')
        gamma_t = cload([128, 8], F32, gamma_d, 'c_gamma')
        beta_t = cload([128, 8], F32, beta_d, 'c_beta')
        bq_t = cload([128, 8], F32, bq_d, 'c_bq')
        bk_t = cload([128, 8], F32, bk_d, 'c_bk')
        bv_t = cload([128, C], F32, bv_d, 'c_bv')
        bfc_t = cload([128, 32], F32, bfc_d, 'c_bfc')
        bfc2_t = cload([128, 8], F32, bfc2_d, 'c_bfc2')
        bcol2_t = cload([128, 8], F32, bcol_d, 'c_bcol')
        ones_t = cload([128, 16], BF16, ones_d, 'c_ones')

        xT_r = _r128(xT_d[:])      # [128, 8, 2048]
        xqb_r = _r128(xqb_d[:])    # [128, 8, 512]
        yT_r = _r128(yT_d[:])      # [128, 8, 512]

        # attnT outlives kqv (written in B, read in C); pools pop LIFO so it
        # opens first and closes at TileContext exit. Tile created lazily at
        # first use (phase B) so it doesn't occupy SBUF during phase A.
        attnT_pool = top.enter_context(tc.tile_pool(name='attnT', bufs=1))

        # K/Q/V buffers live through phases A+B
        es_kqv = ExitStack()
        kqv = es_kqv.enter_context(tc.tile_pool(name='kqv', bufs=1))
        K_bf = kqv.tile([128, 8, T], BF16)            # K^T
        Q_bf = kqv.tile([128, 8, TQ], BF16)           # Q^T
        V_bf = kqv.tile([128, 16, H, D + 1], BF16)    # token-major V + ones col

        # ================= Phase A: DyT + QKV projections =================
        with (
            tc.tile_pool(name='hT_pool', bufs=1) as hpool,
            tc.tile_pool(name='stageA', bufs=2) as spool,
            tc.tile_pool(name='wA', bufs=8) as wpool,
            tc.tile_pool(name='wvA', bufs=1) as wvpool,
            tc.tile_pool(name='psA', bufs=4, space='PSUM') as psA,
        ):
            hT = hpool.tile([128, 8, T], F32R)
            # DyT with gamma/beta folded into the weights host-side:
            # hT = tanh(alpha * x), batched 4 kt-chunks per op.
            # nt-outer so K-proj's first (mt, nt=0) group unblocks early.
            for nt in range(4):
                for k4 in range(2):
                    xt = spool.tile([128, 4, TQ], F32, tag='xstage')
                    nc.sync.dma_start(
                        xt[:], xT_r[:, k4 * 4:(k4 + 1) * 4, nt * TQ:(nt + 1) * TQ])
                    nc.scalar.activation(
                        hT[:, k4 * 4:(k4 + 1) * 4, nt * TQ:(nt + 1) * TQ],
                        xt[:], AF.Tanh, scale=alpha_t[:, 0:1])

            wv_r = _r128(wv_d[:])

            # K^T = wk^T @ hT  (+bk)
            for mt in range(8):
                wt = wpool.tile([128, 8, 128], F32R, tag='wkq')
                nc.sync.dma_start(wt[:], wk_d[:, mt])
                for nt in range(4):
                    ps = psA.tile([128, TQ], F32)
                    for kt in range(8):
                        nc.tensor.matmul(ps[:], wt[:, kt, :], hT[:, kt, nt * TQ:(nt + 1) * TQ],
                                         start=(kt == 0), stop=(kt == 7))
                    nc.scalar.activation(K_bf[:, mt, nt * TQ:(nt + 1) * TQ], ps[:],
                                         AF.Identity, bias=bk_t[:, mt:mt + 1])

            # Q^T = wq^T @ hT[:, :512]  (+bq)
            for mt in range(8):
                wt = wpool.tile([128, 8, 128], F32R, tag='wkq')
                nc.sync.dma_start(wt[:], wq_d[:, mt])
                ps = psA.tile([128, TQ], F32)
                for kt in range(8):
                    nc.tensor.matmul(ps[:], wt[:, kt, :], hT[:, kt, 0:TQ],
                                     start=(kt == 0), stop=(kt == 7))
                nc.scalar.activation(Q_bf[:, mt, :], ps[:], AF.Identity,
                                     bias=bq_t[:, mt:mt + 1])

            # V = hT^T @ wv (token-major) (+bv), into [128, kvb, head, 65] with ones col
            for n2 in range(2):
                wvt = wvpool.tile([128, 8, TQ], F32R, tag='wv')
                nc.sync.dma_start(wvt[:], wv_r[:, :, n2 * TQ:(n2 + 1) * TQ])
                for kvb in range(16):
                    ps = psA.tile([128, TQ], F32)
                    for kt in range(8):
                        nc.tensor.matmul(ps[:], hT[:, kt, kvb * 128:(kvb + 1) * 128],
                                         wvt[:, kt, :],
                                         start=(kt == 0), stop=(kt == 7))
                    bvb = bv_t[:, n2 * TQ:(n2 + 1) * TQ].rearrange(
                        "p (h d) -> p h d", d=D)
                    nc.vector.tensor_tensor(
                        V_bf[:, kvb, n2 * 8:(n2 + 1) * 8, 0:D],
                        ps[:].rearrange("p (h d) -> p h d", d=D),
                        bvb, ALU.add)
            for kvb in range(16):
                nc.vector.tensor_copy(V_bf[:, kvb, :, D], ones_t[:, :])

        # ================= Phase B: attention =================
        with (
            tc.tile_pool(name='pB', bufs=4) as pbpool,
            tc.tile_pool(name='mtriB', bufs=1) as mtpool,
            tc.tile_pool(name='psS', bufs=3, space='PSUM') as psS,
            tc.tile_pool(name='psO', bufs=2, space='PSUM') as psO,
        ):
            mtri_t = mtpool.tile([128, 4, TQ], F32)
            nc.sync.dma_start(mtri_t[:], mtri_d[:])
            attnT = attnT_pool.tile([128, 8, TQ], F32R)
            for h in range(H if 'B' in phases else 0):
                hb = (h % 2) * 64
                hc = h // 2
                po = psO.tile([65, TQ], F32, tag='po')
                for kv2 in range(8):
                    # two kv blocks share one PSUM tile so exp runs [128, 1024]
                    ps = psS.tile([128, 2, TQ], F32, tag='score')
                    pt = pbpool.tile([128, 2, TQ], BF16, tag='probs')
                    for j in range(2):
                        kvb = kv2 * 2 + j
                        nc.tensor.matmul(ps[:, j, :],
                                         K_bf[hb:hb + 64, hc, kvb * 128:(kvb + 1) * 128],
                                         Q_bf[hb:hb + 64, hc, :],
                                         start=True, stop=True)
                        if kvb < 4:
                            nc.vector.tensor_tensor(ps[:, j, :], ps[:, j, :],
                                                    mtri_t[:, kvb, :], ALU.add)
                    nc.scalar.activation(
                        pt[:], ps[:], AF.Exp,
                        bias=bcol2_t[:, kv2:kv2 + 1], scale=0.125)
                    for j in range(2):
                        kvb = kv2 * 2 + j
                        nc.tensor.matmul(po[:], V_bf[:, kvb, h, :], pt[:, j, :],
                                         start=(kvb == 0), stop=(kvb == 15))
                rec = pbpool.tile([1, TQ], F32, tag='recip')
                nc.vector.reciprocal(rec[:], po[64:65, :])
                rec64 = pbpool.tile([64, TQ], F32, tag='recip64')
                nc.gpsimd.partition_broadcast(rec64[:], rec[0:1, :])
                nc.vector.tensor_tensor(attnT[hb:hb + 64, hc, :], po[0:64, :],
                                        rec64[:], ALU.mult)
        es_kqv.close()

        # x2T/h2T live through phases C+D
        es_mlp = ExitStack()
        mpool = es_mlp.enter_context(tc.tile_pool(name='mlp', bufs=1))
        x2T = mpool.tile([128, 8, TQ], F32)
        h2T = mpool.tile([128, 8, TQ], F32R)

        # ================= Phase C: proj + residual + DyT2 =================
        with (
            tc.tile_pool(name='stageC', bufs=3) as scpool,
            tc.tile_pool(name='xqbC', bufs=1) as xqpool,
            tc.tile_pool(name='wC', bufs=16) as wcpool,
            tc.tile_pool(name='psC', bufs=4, space='PSUM') as psC,
        ):
            xqb_t = xqpool.tile([128, 8, TQ], F32)
            nc.sync.dma_start(xqb_t[:], xqb_r[:])
            for mt in range(8 if 'C' in phases else 0):
                wt = wcpool.tile([128, 8, 128], F32R, tag='wproj')
                nc.sync.dma_start(wt[:], wproj_d[:, mt])
                ps = psC.tile([128, TQ], F32)
                for kt in range(8):
                    nc.tensor.matmul(ps[:], wt[:, kt, :], attnT[:, kt, :],
                                     start=(kt == 0), stop=(kt == 7))
                nc.vector.tensor_tensor(x2T[:, mt, :], ps[:], xqb_t[:, mt, :], ALU.add)
                nc.scalar.activation(h2T[:, mt, :], x2T[:, mt, :], AF.Tanh,
                                     scale=alpha_t[:, 0:1])

        # ================= Phase D: MLP =================
        with (
            tc.tile_pool(name='gT_pool', bufs=1) as gpool,
            tc.tile_pool(name='stageD', bufs=3) as sdpool,
            tc.tile_pool(name='wD', bufs=16) as wdpool,
            tc.tile_pool(name='psD', bufs=6, space='PSUM') as psD,
        ):
            gT = gpool.tile([128, 32, TQ], F32R)
            for mt in range(32 if 'D' in phases else 0):
                wt = wdpool.tile([128, 8, 128], F32R, tag='wfc')
                nc.sync.dma_start(wt[:], wfc_d[:, mt])
                ps = psD.tile([128, TQ], F32)
                for kt in range(8):
                    nc.tensor.matmul(ps[:], wt[:, kt, :], h2T[:, kt, :],
                                     start=(kt == 0), stop=(kt == 7))
                nc.scalar.activation(gT[:, mt, :], ps[:], AF.Gelu,
                                     bias=bfc_t[:, mt:mt + 1])

            for mt in range(8 if 'D' in phases else 0):
                wt = wdpool.tile([128, 32, 128], F32R, tag='wfc2')
                nc.sync.dma_start(wt[:], wfc2_d[:, mt])
                ps = psD.tile([128, TQ], F32)
                for kt in range(32):
                    nc.tensor.matmul(ps[:], wt[:, kt, :], gT[:, kt, :],
                                     start=(kt == 0), stop=(kt == 31))
                tmp = sdpool.tile([128, TQ], F32, tag='bias2')
                nc.vector.tensor_scalar(tmp[:], ps[:], bfc2_t[:, mt:mt + 1], None, ALU.add)
                yt = sdpool.tile([128, TQ], F32, tag='yout')
                nc.vector.tensor_tensor(yt[:], tmp[:], x2T[:, mt, :], ALU.add)
                nc.sync.dma_start(yT_r[:, mt, :], yt[:])
        es_mlp.close()

    nc.finalize()
    return nc


def _prep_inputs(x, alpha, gamma, beta, w_attn, b_attn, w_proj, b_proj,
                 w_fc, b_fc, w_fc2, b_fc2):
    f = np.float32

    def tile_w(w, n_mt):
        # [K, M] -> [128, mt, kt, 128]: element [p, mt, kt, c] = w[kt*128+p, mt*128+c]
        kk, mm = w.shape
        return np.ascontiguousarray(
            np.asarray(w, f).reshape(kk // 128, 128, n_mt, 128).transpose(1, 2, 0, 3))

    # Fold DyT's gamma/beta into the consuming weights:
    #   w.T @ (g*t + b) = (g[:,None]*w).T @ t + (w.T @ b)
    g64 = np.asarray(gamma, np.float64)
    b64 = np.asarray(beta, np.float64)
    w64 = np.asarray(w_attn, np.float64)
    wfc64 = np.asarray(w_fc, np.float64)
    wq64, wk64, wv64 = w64[:, :C], w64[:, C:2 * C], w64[:, 2 * C:]
    bq_e = np.asarray(b_attn[:C], np.float64) + wq64.T @ b64
    bk_e = np.asarray(b_attn[C:2 * C], np.float64) + wk64.T @ b64
    bv_e = np.asarray(b_attn[2 * C:], np.float64) + wv64.T @ b64
    bfc_e = np.asarray(b_fc, np.float64) + wfc64.T @ b64

    wq = tile_w(wq64 * g64[:, None], 8)
    wk = tile_w(wk64 * g64[:, None], 8)
    wv = np.ascontiguousarray(wv64 * g64[:, None], f)
    bq = np.ascontiguousarray(bq_e.reshape(8, 128).T, f)
    bk = np.ascontiguousarray(bk_e.reshape(8, 128).T, f)
    bv = np.ascontiguousarray(np.tile(bv_e.reshape(1, C), (128, 1)), f)
    bfc = np.ascontiguousarray(bfc_e.reshape(32, 128).T, f)
    bfc2 = np.ascontiguousarray(b_fc2.reshape(8, 128).T, f)
    alpha_b = np.full((128, 1), float(np.asarray(alpha).reshape(-1)[0]), f)
    gamma_c = np.ascontiguousarray(np.asarray(gamma, f).reshape(8, 128).T, f)
    beta_c = np.ascontiguousarray(np.asarray(beta, f).reshape(8, 128).T, f)
    r = np.arange(128)[:, None, None]
    tt = np.arange(4)[None, :, None]
    p = np.arange(TQ)[None, None, :]
    mask_tri = np.where(tt * 128 + r <= p, 0.0, NEG).astype(f)
    ones_bf = np.ones((128, 16), ml_dtypes.bfloat16)

    shared = dict(wq=wq, wk=wk, wv=wv, wproj=tile_w(w_proj, 8),
                  wfc=tile_w(wfc64 * g64[:, None], 32),
                  wfc2=tile_w(w_fc2, 8),
                  bq=bq, bk=bk, bv=bv, bfc=bfc, bfc2=bfc2,
                  alpha_b=alpha_b, gamma_c=gamma_c, beta_c=beta_c,
                  mask_tri=mask_tri, ones_bf=ones_bf)

    in_maps = []
    for c in range(8):
        b, qs = c // 4, c % 4
        perm = np.concatenate([np.arange(qs * TQ, (qs + 1) * TQ),
                               np.arange(0, qs * TQ),
                               np.arange((qs + 1) * TQ, T)])
        xT = np.ascontiguousarray(np.asarray(x[b], f).T[:, perm])
        xqb = np.ascontiguousarray(np.asarray(x[b, qs * TQ:(qs + 1) * TQ], f).T
                                   + np.asarray(b_proj, f)[:, None])
        bias_cols = np.zeros((128, 8), f)
        bias_cols[:, 2 + 2 * qs:] = NEG
        in_maps.append(dict(shared, xT=xT, xqb=xqb, bias_cols=bias_cols))
    return in_maps


def kernel(**inputs):
    if 'nc' not in _CACHE:
        _CACHE['nc'] = _build()
    nc = _CACHE['nc']
    in_maps = _prep_inputs(**inputs)
    res = run_bass_kernel_spmd(nc, in_maps, core_ids=list(range(8)))
    out = np.zeros((2, T, C), np.float32)
    for c in range(8):
        b, qs = c // 4, c % 4
        out[b, qs * TQ:(qs + 1) * TQ, :] = res.results[c]['yT'].T
    return out
